# revision 30
# baseline (speedup 1.0000x reference)
"""Multi-head self-attention with tanh logit softcap + key masking, on 8 trn2 cores.

Strategy:
- Data-parallel over batch: core b handles batch b (B=8).
- Sparsity: masked keys (mask=True) are removed on the host; K/V are projected
  from a compacted Qc (unmasked rows only, zero-padded to M_pad). Pad columns
  are excluded from the softmax denominator via the padmask columns of the
  combined PV lhsT, and contribute zero to the numerator because their V rows
  are exactly zero.
- Head padding: the 8 heads (D=16) are padded to 32-row strips (2 groups of 4
  heads) via host-permuted projection weights, so score matmuls (K=16->32) can
  use 4-way tile_position row tiling and PV matmuls use col tiling.
- Scores are computed transposed (U^T[m, n]) so that P^T = exp(clip*tanh(U^T/4))
  feeds the PV matmuls (contraction over m on the partition dim) directly.
- PV lhsT = [v | padmask] (M=32): rows 0:16 of each strip accumulate the
  numerator, rows 16:32 the (replicated) denominator. An S matmul gathers the
  denominator rows and broadcasts their reciprocals to all 128 partitions for
  a single full-tile normalize multiply. The output projection uses
  host-padded Wvout rows (zeros kill the denominator/garbage rows).
- float32r (single-pass reduced-precision fp32) for the score/projection
  matmuls; PV matmuls stay fp32 (f32r rejects col-tiled tile_position).
"""

import functools
from contextlib import ExitStack

import numpy as np

import concourse.bass as bass
import concourse.mybir as mybir
import concourse.tile as tile
from concourse import bacc
from concourse import bass_utils
from concourse.tile import add_dep_helper

F32 = mybir.dt.float32
F32R = mybir.dt.float32r
AF = mybir.ActivationFunctionType

N = 1024
E = 128
H = 8
D = 16
NCORES = 8
NW = 512  # query-chunk width
NJ = N // NW  # number of query chunks

# S_BC[src, dst] = 1 iff src is the denominator row of dst's 32-strip:
# out[dst, n] = rdsb[32*(dst//32)+16, n] (gather+broadcast via matmul).
S_BC = np.zeros((128, 128), np.float32)
for _dst in range(128):
    S_BC[32 * (_dst // 32) + 16, _dst] = 1.0


def _pad_weight_in(W):
    """[E, H*D] -> [E, 2*128]: head h -> group h//4, strip 32*(h%4), cols 16:32 zero."""
    Wp = np.zeros((E, 2, 128), np.float32)
    for h in range(H):
        g, hp = divmod(h, 4)
        Wp[:, g, 32 * hp : 32 * hp + D] = W[:, D * h : D * (h + 1)]
    return np.ascontiguousarray(Wp.reshape(E, 256))


def _pad_weight_out(W):
    """[H*D, E] -> [128, 2*E] with padded rows (pad rows zero)."""
    Wp = np.zeros((128, 2, E), np.float32)
    for h in range(H):
        g, hp = divmod(h, 4)
        Wp[32 * hp : 32 * hp + D, g, :] = W[D * h : D * (h + 1), :]
    return np.ascontiguousarray(Wp.reshape(128, 2 * E))


@functools.lru_cache(maxsize=4)
def _build(Mp: int, clipf: float):
    MC = Mp // 128
    nc = bacc.Bacc(
        "TRN2",
        target_bir_lowering=False,
        debug=False,
        enable_asserts=True,
        num_devices=NCORES,
    )
    QT_d = nc.dram_tensor("QT", [E, N], F32R, kind="ExternalInput").ap()
    QcT_d = nc.dram_tensor("QcT", [E, Mp], F32R, kind="ExternalInput").ap()
    pm_d = nc.dram_tensor("pm32", [Mp, 32], F32, kind="ExternalInput").ap()
    wq_d = nc.dram_tensor("Wqp", [E, 256], F32R, kind="ExternalInput").ap()
    wk_d = nc.dram_tensor("Wkp", [E, 256], F32R, kind="ExternalInput").ap()
    wvi_d = nc.dram_tensor("Wvip", [E, 256], F32R, kind="ExternalInput").ap()
    wvo_d = nc.dram_tensor("Wvop", [128, 256], F32R, kind="ExternalInput").ap()
    O_d = nc.dram_tensor("O", [N, E], F32, kind="ExternalOutput").ap()

    with tile.TileContext(nc) as tc, ExitStack() as ctx:
        consts = ctx.enter_context(tc.tile_pool(name="consts", bufs=1))
        sbt = ctx.enter_context(tc.tile_pool(name="sbt", bufs=2))
        sbp = ctx.enter_context(tc.tile_pool(name="sbp", bufs=3))
        sbn = ctx.enter_context(tc.tile_pool(name="sbn", bufs=2))
        sbo = ctx.enter_context(tc.tile_pool(name="sbo", bufs=3))
        psq = ctx.enter_context(tc.tile_pool(name="psq", bufs=2, space="PSUM"))
        psp = ctx.enter_context(tc.tile_pool(name="psp", bufs=1, space="PSUM"))
        pvo = ctx.enter_context(tc.tile_pool(name="pvo", bufs=1, space="PSUM"))

        # ---------- loads ----------
        wtile = consts.tile([128, 1], F32)
        nc.vector.memset(wtile, 0.0)
        # warmup activation: forces the ACT table load to overlap input DMAs
        nc.scalar.activation(wtile, wtile, AF.Tanh, scale=0.25)

        wq = consts.tile([E, 256], F32R)
        nc.sync.dma_start(out=wq, in_=wq_d)
        wk = consts.tile([E, 256], F32R)
        nc.scalar.dma_start(out=wk, in_=wk_d)
        qt = consts.tile([E, N], F32R)
        nc.sync.dma_start(out=qt[:, 0:512], in_=QT_d[:, 0:512])
        qct = consts.tile([E, Mp], F32R)
        nc.scalar.dma_start(out=qct[:, 0 : min(512, Mp)], in_=QcT_d[:, 0 : min(512, Mp)])
        nc.sync.dma_start(out=qt[:, 512:], in_=QT_d[:, 512:])
        if Mp > 512:
            nc.scalar.dma_start(out=qct[:, 512:], in_=QcT_d[:, 512:])
        wvi = consts.tile([E, 256], F32R)
        nc.gpsimd.dma_start(out=wvi, in_=wvi_d)
        pm32 = consts.tile([128, MC, 32], F32)
        nc.gpsimd.dma_start(out=pm32, in_=pm_d.rearrange("(c p) w -> p c w", p=128))
        pm = consts.tile([128, MC, 128], F32)
        pmr = pm.rearrange("p c (s w) -> p c s w", s=4)
        for s4 in range(4):
            nc.vector.tensor_copy(pmr[:, :, s4, :], pm32)
        # S_BC[src, dst] = 1 iff src == 32*(dst//32)+16, built via affine_select
        sbc = consts.tile([128, 4, 32], F32)
        nc.gpsimd.memset(sbc, 0.0)
        nc.gpsimd.affine_select(
            out=sbc,
            in_=sbc,
            compare_op=mybir.AluOpType.not_equal,
            fill=1.0,
            base=-16,
            pattern=[[-32, 4], [0, 32]],
            channel_multiplier=1,
        )
        sbc = sbc.rearrange("p a b -> p (a b)")
        wvo = consts.tile([128, 256], F32R)
        nc.gpsimd.dma_start(out=wvo, in_=wvo_d)

        dz = consts.tile([128, NW], F32)
        nc.vector.memset(dz, 0.0)
        dzr = consts.tile([128, NW], F32R)
        nc.vector.tensor_copy(dzr, dz)

        qTp = [consts.tile([128, N], F32R, name=f"qTp{g}") for g in range(2)]
        kTp = [consts.tile([128, Mp], F32R, name=f"kTp{g}") for g in range(2)]
        vp = [consts.tile([128, MC, 128], F32, name=f"vp{g}") for g in range(2)]

        # ---------- projection emitters (interleaved into the main loop) ----------
        def emit_qproj(j):
            js = slice(j * 512, (j + 1) * 512)
            for g in range(2):
                gs = slice(g * 128, (g + 1) * 128)
                pq = psq.tile([128, 512], F32, tag="proj", name="pq")
                nc.tensor.matmul(pq, lhsT=wq[:, gs], rhs=qt[:, js], start=True, stop=True)
                nc.vector.tensor_copy(qTp[g][:, js], pq)

        def emit_kproj(s, w):
            for g in range(2):
                gs = slice(g * 128, (g + 1) * 128)
                pk = psq.tile([128, 512], F32, tag="proj", name="pk")
                nc.tensor.matmul(
                    pk[:, :w], lhsT=wk[:, gs], rhs=qct[:, s : s + w], start=True, stop=True
                )
                nc.vector.tensor_copy(kTp[g][:, s : s + w], pk[:, :w])

        def emit_vproj(c):
            cs = slice(c * 128, (c + 1) * 128)
            for g in range(2):
                gs = slice(g * 128, (g + 1) * 128)
                pv_ = psq.tile([128, 512], F32, tag="proj", name="pv_")
                nc.tensor.matmul(
                    pv_[:, :128], lhsT=qct[:, cs], rhs=wvi[:, gs], start=True, stop=True
                )
                nc.vector.tensor_add(vp[g][:, c, :], pv_[:, :128], pm[:, c, :])

        emit_qproj(0)
        emit_kproj(0, 128)
        emit_kproj(128, min(512, Mp) - 128)
        emit_vproj(0)
        # deferred prologue pieces, emitted after given (j, mc) iterations
        deferred = {}
        for c in range(1, MC):
            deferred.setdefault((0, c - 1), []).append(lambda c=c: emit_vproj(c))
        deferred.setdefault((0, 0), []).append(lambda: emit_qproj(1))
        if Mp > 512:
            deferred.setdefault((0, 1), []).append(lambda: emit_kproj(512, Mp - 512))

        # ---------- main loop over query chunks ----------
        o_accs = {}

        def emit_epilogue(j):
            o_acc = o_accs[j]
            osb = sbn.tile([128, 2, NW], F32, tag="osb", name="osb")
            nc.scalar.copy(osb, o_acc)
            rdsb = sbn.tile([128, 2, NW], F32, tag="rdsb", name="rdsb")
            nc.vector.reciprocal_approx_fast(out=rdsb, in_=o_acc)
            rd_b = pvo.tile([128, 2, NW], F32, tag="oacc", name="rd_b")
            for g in range(2):
                nc.tensor.matmul(
                    rd_b[:, g, :], lhsT=sbc, rhs=rdsb[:, g, :], start=True, stop=True
                )
            outn = sbn.tile([128, 2, NW], F32R, tag="outn", name="outn")
            nc.vector.tensor_mul(outn, osb, rd_b)

            for q in range(NW // 128):
                pf_full = pvo.tile([128, 2, NW], F32, tag="oacc", name="pf_full")
                pf = pf_full[:, 0, :128]
                pf0 = None
                for g in range(2):
                    mm = nc.tensor.matmul(
                        pf,
                        lhsT=outn[:, g, q * 128 : (q + 1) * 128],
                        rhs=wvo[:, g * 128 : (g + 1) * 128],
                        start=(g == 0),
                        stop=(g == 1),
                    )
                    if g == 0:
                        pf0 = mm
                    else:
                        add_dep_helper(
                            mm.ins, pf0.ins, sync=False, reason="proj accum order"
                        )
                fout = sbo.tile([128, 128], F32, tag="fout", name="fout")
                nc.vector.tensor_copy(fout, pf)
                nc.sync.dma_start(
                    out=O_d[j * NW + q * 128 : j * NW + (q + 1) * 128, :], in_=fout
                )

        for j in range(NJ):
            js = slice(j * NW, (j + 1) * NW)
            first_pv = {}

            for c in range(MC):
                cs = slice(c * 128, (c + 1) * 128)
                t = sbt.tile([128, 2, 4, NW], F32, tag="t", name="t")
                p = sbp.tile([128, 2, 4, NW], F32, tag="p", name="p")
                last = c == MC - 1
                for g in range(2):
                    ps = psp.tile([128, 4, NW], F32, tag="scores", name="ps")
                    for hp in range(4):
                        rs = slice(32 * hp, 32 * (hp + 1))
                        nc.tensor.matmul(
                            ps[:, hp, :],
                            lhsT=kTp[g][rs, cs],
                            rhs=qTp[g][rs, js],
                            start=True,
                            stop=True,
                            tile_position=(32 * hp, 0),
                        )
                    nc.scalar.activation(t[:, g], ps, AF.Tanh, scale=0.25)
                if c == 0:
                    # previous chunk's epilogue goes after this chunk's ACT work
                    # so ACT keeps streaming while PE runs the epilogue.
                    if j > 0:
                        emit_epilogue(j - 1)
                    o_acc = pvo.tile([128, 2, NW], F32, tag="oacc", name="o_acc")
                    o_accs[j] = o_acc
                    # dummy zeroing matmuls: start each psum bank's accumulation
                    # group with a full-partition write so the col-tiled M=32
                    # matmuls below can accumulate (has_written set everywhere).
                    for g in range(2):
                        first_pv[g] = nc.tensor.matmul(
                            o_acc[:, g, :], lhsT=dzr[:, 0:128], rhs=dzr, start=True,
                            stop=False, skip_group_check=True,
                        )
                for g in range(2):
                    nc.scalar.activation(p[:, g], t[:, g], AF.Exp, scale=clipf)
                    for hp in range(4):
                        cols = slice(32 * hp, 32 * (hp + 1))
                        mo = nc.tensor.matmul(
                            o_acc[cols, g, :],
                            lhsT=vp[g][:, c, cols],
                            rhs=p[:, g, hp, :],
                            start=False,
                            stop=last,
                            tile_position=(0, 32 * hp),
                            skip_group_check=True,
                        )
                        add_dep_helper(
                            mo.ins,
                            first_pv[g].ins,
                            sync=False,
                            reason="psum group start",
                        )
                for fn in deferred.pop((j, c), []):
                    fn()

        emit_epilogue(NJ - 1)

    nc.compile()
    return nc


def _prep(Q, Wq, Wk, Wvin, Wvout, mask, clip):
    Q = np.asarray(Q, dtype=np.float32)
    Wq = np.asarray(Wq, dtype=np.float32)
    Wk = np.asarray(Wk, dtype=np.float32)
    Wvin = np.asarray(Wvin, dtype=np.float32)
    Wvout = np.asarray(Wvout, dtype=np.float32)
    mask = np.asarray(mask)
    clipf = float(np.asarray(clip))
    B = Q.shape[0]

    counts = (~mask).sum(axis=1)
    Mp = max(128, int(-(-int(counts.max()) // 128) * 128))

    wq_p = _pad_weight_in(Wq)
    wk_p = _pad_weight_in(Wk)
    wvi_p = _pad_weight_in(Wvin)
    wvo_p = _pad_weight_out(Wvout)

    in_maps = []
    for b in range(B):
        idx = np.nonzero(~mask[b])[0]
        Qc = np.zeros((Mp, E), np.float32)
        Qc[: len(idx)] = Q[b, idx]
        pmv = np.zeros((Mp, 1), np.float32)
        pmv[: len(idx)] = 1.0
        pm32 = np.zeros((Mp, 32), np.float32)
        pm32[:, 16:32] = pmv
        in_maps.append(
            {
                "QT": np.ascontiguousarray(Q[b].T),
                "QcT": np.ascontiguousarray(Qc.T),
                "pm32": pm32,
                "Wqp": wq_p,
                "Wkp": wk_p,
                "Wvip": wvi_p,
                "Wvop": wvo_p,
            }
        )
    return Mp, clipf, in_maps


def kernel(Q, Wq, Wk, Wvin, Wvout, mask, clip):
    Mp, clipf, in_maps = _prep(Q, Wq, Wk, Wvin, Wvout, mask, clip)
    nc = _build(Mp, clipf)
    res = bass_utils.run_bass_kernel_spmd(nc, in_maps, core_ids=list(range(NCORES)))
    return np.stack([res.results[c]["O"] for c in range(NCORES)])


def kernel_traced(Q, Wq, Wk, Wvin, Wvout, mask, clip):
    """Like kernel() but with NTFF tracing; returns (output, exec_time_ns)."""
    Mp, clipf, in_maps = _prep(Q, Wq, Wk, Wvin, Wvout, mask, clip)
    nc = _build(Mp, clipf)
    res = bass_utils.run_bass_kernel_spmd(
        nc, in_maps, core_ids=list(range(NCORES)), trace=True
    )
    out = np.stack([res.results[c]["O"] for c in range(NCORES)])
    return out, res.exec_time_ns


# revision 31
# speedup vs baseline: 1.1090x; 1.1090x over previous
"""Multi-head self-attention with tanh logit softcap + key masking, on 8 trn2 cores.

Strategy:
- Data-parallel over batch: core b handles batch b (B=8).
- Sparsity: masked keys (mask=True) are removed on the host; K/V are projected
  from a compacted Qc (unmasked rows only, zero-padded to M_pad). Pad columns
  are excluded from the softmax denominator via the padmask columns of the
  combined PV lhsT, and contribute zero to the numerator because their V rows
  are exactly zero.
- Head padding: the 8 heads (D=16) are padded to 32-row strips (2 groups of 4
  heads) via host-permuted projection weights, so score matmuls (K=16->32) can
  use 4-way tile_position row tiling and PV matmuls use col tiling.
- Scores are computed transposed (U^T[m, n]) so that P^T = exp(clip*tanh(U^T/4))
  feeds the PV matmuls (contraction over m on the partition dim) directly.
- PV lhsT = [v | padmask] (M=32): rows 0:16 of each strip accumulate the
  numerator, rows 16:32 the (replicated) denominator. An S matmul gathers the
  denominator rows and broadcasts their reciprocals to all 128 partitions for
  a single full-tile normalize multiply. The output projection uses
  host-padded Wvout rows (zeros kill the denominator/garbage rows).
- float32r (single-pass reduced-precision fp32) for the score/projection
  matmuls; PV matmuls stay fp32 (f32r rejects col-tiled tile_position).
"""

import functools
from contextlib import ExitStack

import numpy as np

import concourse.bass as bass
import concourse.mybir as mybir
import concourse.tile as tile
from concourse import bacc
from concourse import bass_utils
from concourse.tile import add_dep_helper

F32 = mybir.dt.float32
F32R = mybir.dt.float32r
AF = mybir.ActivationFunctionType

N = 1024
E = 128
H = 8
D = 16
NCORES = 8
NW = 512  # query-chunk width
NJ = N // NW  # number of query chunks

# S_BC[src, dst] = 1 iff src is the denominator row of dst's 32-strip:
# out[dst, n] = rdsb[32*(dst//32)+16, n] (gather+broadcast via matmul).
S_BC = np.zeros((128, 128), np.float32)
for _dst in range(128):
    S_BC[32 * (_dst // 32) + 16, _dst] = 1.0


def _pad_weight_in(W):
    """[E, H*D] -> [E, 2*128]: head h -> group h//4, strip 32*(h%4), cols 16:32 zero."""
    Wp = np.zeros((E, 2, 128), np.float32)
    for h in range(H):
        g, hp = divmod(h, 4)
        Wp[:, g, 32 * hp : 32 * hp + D] = W[:, D * h : D * (h + 1)]
    return np.ascontiguousarray(Wp.reshape(E, 256))


def _pad_weight_out(W):
    """[H*D, E] -> [128, 2*E] with padded rows (pad rows zero)."""
    Wp = np.zeros((128, 2, E), np.float32)
    for h in range(H):
        g, hp = divmod(h, 4)
        Wp[32 * hp : 32 * hp + D, g, :] = W[D * h : D * (h + 1), :]
    return np.ascontiguousarray(Wp.reshape(128, 2 * E))


@functools.lru_cache(maxsize=4)
def _build(Mp: int, clipf: float):
    MC = Mp // 128
    nc = bacc.Bacc(
        "TRN2",
        target_bir_lowering=False,
        debug=False,
        enable_asserts=True,
        num_devices=NCORES,
    )
    QT_d = nc.dram_tensor("QT", [E, N], F32R, kind="ExternalInput").ap()
    QcT_d = nc.dram_tensor("QcT", [E, Mp], F32R, kind="ExternalInput").ap()
    pm_d = nc.dram_tensor("pm32", [Mp, 32], F32, kind="ExternalInput").ap()
    wq_d = nc.dram_tensor("Wqp", [E, 256], F32R, kind="ExternalInput").ap()
    wk_d = nc.dram_tensor("Wkp", [E, 256], F32R, kind="ExternalInput").ap()
    wvi_d = nc.dram_tensor("Wvip", [E, 256], F32R, kind="ExternalInput").ap()
    wvo_d = nc.dram_tensor("Wvop", [128, 256], F32R, kind="ExternalInput").ap()
    O_d = nc.dram_tensor("O", [N, E], F32, kind="ExternalOutput").ap()

    with tile.TileContext(nc) as tc, ExitStack() as ctx:
        consts = ctx.enter_context(tc.tile_pool(name="consts", bufs=1))
        sbt = ctx.enter_context(tc.tile_pool(name="sbt", bufs=2))
        sbp = ctx.enter_context(tc.tile_pool(name="sbp", bufs=3))
        sbn = ctx.enter_context(tc.tile_pool(name="sbn", bufs=2))
        sbo = ctx.enter_context(tc.tile_pool(name="sbo", bufs=3))
        psq = ctx.enter_context(tc.tile_pool(name="psq", bufs=2, space="PSUM"))
        psp = ctx.enter_context(tc.tile_pool(name="psp", bufs=1, space="PSUM"))
        pvo = ctx.enter_context(tc.tile_pool(name="pvo", bufs=1, space="PSUM"))

        # ---------- loads ----------
        wtile = consts.tile([128, 1], F32)
        nc.vector.memset(wtile, 0.0)
        # warmup activation: forces the ACT table load to overlap input DMAs
        nc.scalar.activation(wtile, wtile, AF.Tanh, scale=0.25)

        wq = consts.tile([E, 256], F32R)
        nc.sync.dma_start(out=wq, in_=wq_d)
        wk = consts.tile([E, 256], F32R)
        nc.scalar.dma_start(out=wk, in_=wk_d)
        qt = consts.tile([E, N], F32R)
        nc.sync.dma_start(out=qt[:, 0:512], in_=QT_d[:, 0:512])
        qct = consts.tile([E, Mp], F32R)
        nc.scalar.dma_start(out=qct[:, 0 : min(512, Mp)], in_=QcT_d[:, 0 : min(512, Mp)])
        nc.sync.dma_start(out=qt[:, 512:], in_=QT_d[:, 512:])
        if Mp > 512:
            nc.scalar.dma_start(out=qct[:, 512:], in_=QcT_d[:, 512:])
        wvi = consts.tile([E, 256], F32R)
        nc.gpsimd.dma_start(out=wvi, in_=wvi_d)
        pm32 = consts.tile([128, MC, 32], F32)
        nc.gpsimd.dma_start(out=pm32, in_=pm_d.rearrange("(c p) w -> p c w", p=128))
        pm = consts.tile([128, MC, 128], F32)
        pmr = pm.rearrange("p c (s w) -> p c s w", s=4)
        for s4 in range(4):
            nc.vector.tensor_copy(pmr[:, :, s4, :], pm32)
        # S_BC[src, dst] = 1 iff src == 32*(dst//32)+16, built via affine_select
        sbc = consts.tile([128, 4, 32], F32)
        nc.gpsimd.memset(sbc, 0.0)
        nc.gpsimd.affine_select(
            out=sbc,
            in_=sbc,
            compare_op=mybir.AluOpType.not_equal,
            fill=1.0,
            base=-16,
            pattern=[[-32, 4], [0, 32]],
            channel_multiplier=1,
        )
        sbc = sbc.rearrange("p a b -> p (a b)")
        wvo = consts.tile([128, 256], F32R)
        nc.gpsimd.dma_start(out=wvo, in_=wvo_d)

        dz = consts.tile([128, NW], F32)
        nc.vector.memset(dz, 0.0)
        dzr = consts.tile([128, NW], F32R)
        nc.vector.tensor_copy(dzr, dz)

        qTp = [consts.tile([128, N], F32R, name=f"qTp{g}") for g in range(2)]
        kTp = [consts.tile([128, Mp], F32R, name=f"kTp{g}") for g in range(2)]
        vp = [consts.tile([128, MC, 128], F32, name=f"vp{g}") for g in range(2)]

        # ---------- projection emitters (interleaved into the main loop) ----------
        def emit_qproj(j):
            js = slice(j * 512, (j + 1) * 512)
            for g in range(2):
                gs = slice(g * 128, (g + 1) * 128)
                pq = psq.tile([128, 512], F32, tag="proj", name="pq")
                nc.tensor.matmul(pq, lhsT=wq[:, gs], rhs=qt[:, js], start=True, stop=True)
                nc.vector.tensor_copy(qTp[g][:, js], pq)

        def emit_kproj(s, w):
            for g in range(2):
                gs = slice(g * 128, (g + 1) * 128)
                pk = psq.tile([128, 512], F32, tag="proj", name="pk")
                nc.tensor.matmul(
                    pk[:, :w], lhsT=wk[:, gs], rhs=qct[:, s : s + w], start=True, stop=True
                )
                nc.vector.tensor_copy(kTp[g][:, s : s + w], pk[:, :w])

        def emit_vproj(c):
            cs = slice(c * 128, (c + 1) * 128)
            for g in range(2):
                gs = slice(g * 128, (g + 1) * 128)
                pv_ = psq.tile([128, 512], F32, tag="proj", name="pv_")
                nc.tensor.matmul(
                    pv_[:, :128], lhsT=qct[:, cs], rhs=wvi[:, gs], start=True, stop=True
                )
                nc.vector.tensor_add(vp[g][:, c, :], pv_[:, :128], pm[:, c, :])

        emit_qproj(0)
        emit_kproj(0, 128)
        emit_kproj(128, min(512, Mp) - 128)
        emit_vproj(0)
        # deferred prologue pieces, emitted after given (j, mc) iterations
        deferred = {}
        for c in range(1, MC):
            deferred.setdefault((0, c - 1), []).append(lambda c=c: emit_vproj(c))
        deferred.setdefault((0, 0), []).append(lambda: emit_qproj(1))
        if Mp > 512:
            deferred.setdefault((0, 1), []).append(lambda: emit_kproj(512, Mp - 512))

        # ---------- main loop over query chunks ----------
        o_accs = {}

        def emit_epilogue(j):
            o_acc = o_accs[j]
            osb = sbn.tile([128, 2, NW], F32, tag="osb", name="osb")
            nc.scalar.copy(osb, o_acc)
            rdsb = sbn.tile([128, 2, NW], F32, tag="rdsb", name="rdsb")
            nc.vector.reciprocal_approx_fast(out=rdsb, in_=o_acc)
            rd_b = pvo.tile([128, 2, NW], F32, tag="oacc", name="rd_b")
            for g in range(2):
                nc.tensor.matmul(
                    rd_b[:, g, :], lhsT=sbc, rhs=rdsb[:, g, :], start=True, stop=True
                )
            outn = sbn.tile([128, 2, NW], F32R, tag="outn", name="outn")
            nc.vector.tensor_mul(outn, osb, rd_b)

            for q in range(NW // 128):
                pf_full = pvo.tile([128, 2, NW], F32, tag="oacc", name="pf_full")
                pf = pf_full[:, 0, :128]
                pf0 = None
                for g in range(2):
                    mm = nc.tensor.matmul(
                        pf,
                        lhsT=outn[:, g, q * 128 : (q + 1) * 128],
                        rhs=wvo[:, g * 128 : (g + 1) * 128],
                        start=(g == 0),
                        stop=(g == 1),
                    )
                    if g == 0:
                        pf0 = mm
                    else:
                        add_dep_helper(
                            mm.ins, pf0.ins, sync=False, reason="proj accum order"
                        )
                fout = sbo.tile([128, 128], F32, tag="fout", name="fout")
                nc.vector.tensor_copy(fout, pf)
                nc.sync.dma_start(
                    out=O_d[j * NW + q * 128 : j * NW + (q + 1) * 128, :], in_=fout
                )

        for j in range(NJ):
            js = slice(j * NW, (j + 1) * NW)
            first_pv = {}

            for c in range(MC):
                cs = slice(c * 128, (c + 1) * 128)
                t = sbt.tile([128, 2, 4, NW], F32, tag="t", name="t")
                p = sbp.tile([128, 2, 4, NW], F32, tag="p", name="p")
                last = c == MC - 1
                for g in range(2):
                    ps = psp.tile([128, 4, NW], F32, tag="scores", name="ps")
                    for hp in range(4):
                        rs = slice(32 * hp, 32 * (hp + 1))
                        nc.tensor.matmul(
                            ps[:, hp, :],
                            lhsT=kTp[g][rs, cs],
                            rhs=qTp[g][rs, js],
                            start=True,
                            stop=True,
                            tile_position=(32 * hp, 0),
                        )
                    nc.scalar.activation(t[:, g], ps, AF.Tanh, scale=0.25)
                nc.scalar.activation(p, t, AF.Exp, scale=clipf)
                if c == 0:
                    # previous chunk's epilogue goes after this chunk's ACT work
                    # so ACT keeps streaming while PE runs the epilogue.
                    if j > 0:
                        emit_epilogue(j - 1)
                    o_acc = pvo.tile([128, 2, NW], F32, tag="oacc", name="o_acc")
                    o_accs[j] = o_acc
                    # dummy zeroing matmuls: start each psum bank's accumulation
                    # group with a full-partition write so the col-tiled M=32
                    # matmuls below can accumulate (has_written set everywhere).
                    for g in range(2):
                        first_pv[g] = nc.tensor.matmul(
                            o_acc[:, g, :], lhsT=dzr[:, 0:128], rhs=dzr, start=True,
                            stop=False, skip_group_check=True,
                        )
                for g in range(2):
                    for hp in range(4):
                        cols = slice(32 * hp, 32 * (hp + 1))
                        mo = nc.tensor.matmul(
                            o_acc[cols, g, :],
                            lhsT=vp[g][:, c, cols],
                            rhs=p[:, g, hp, :],
                            start=False,
                            stop=last,
                            tile_position=(0, 32 * hp),
                            skip_group_check=True,
                        )
                        add_dep_helper(
                            mo.ins,
                            first_pv[g].ins,
                            sync=False,
                            reason="psum group start",
                        )
                for fn in deferred.pop((j, c), []):
                    fn()

        emit_epilogue(NJ - 1)

    nc.compile()
    return nc


def _prep(Q, Wq, Wk, Wvin, Wvout, mask, clip):
    Q = np.asarray(Q, dtype=np.float32)
    Wq = np.asarray(Wq, dtype=np.float32)
    Wk = np.asarray(Wk, dtype=np.float32)
    Wvin = np.asarray(Wvin, dtype=np.float32)
    Wvout = np.asarray(Wvout, dtype=np.float32)
    mask = np.asarray(mask)
    clipf = float(np.asarray(clip))
    B = Q.shape[0]

    counts = (~mask).sum(axis=1)
    Mp = max(128, int(-(-int(counts.max()) // 128) * 128))

    wq_p = _pad_weight_in(Wq)
    wk_p = _pad_weight_in(Wk)
    wvi_p = _pad_weight_in(Wvin)
    wvo_p = _pad_weight_out(Wvout)

    in_maps = []
    for b in range(B):
        idx = np.nonzero(~mask[b])[0]
        Qc = np.zeros((Mp, E), np.float32)
        Qc[: len(idx)] = Q[b, idx]
        pmv = np.zeros((Mp, 1), np.float32)
        pmv[: len(idx)] = 1.0
        pm32 = np.zeros((Mp, 32), np.float32)
        pm32[:, 16:32] = pmv
        in_maps.append(
            {
                "QT": np.ascontiguousarray(Q[b].T),
                "QcT": np.ascontiguousarray(Qc.T),
                "pm32": pm32,
                "Wqp": wq_p,
                "Wkp": wk_p,
                "Wvip": wvi_p,
                "Wvop": wvo_p,
            }
        )
    return Mp, clipf, in_maps


def kernel(Q, Wq, Wk, Wvin, Wvout, mask, clip):
    Mp, clipf, in_maps = _prep(Q, Wq, Wk, Wvin, Wvout, mask, clip)
    nc = _build(Mp, clipf)
    res = bass_utils.run_bass_kernel_spmd(nc, in_maps, core_ids=list(range(NCORES)))
    return np.stack([res.results[c]["O"] for c in range(NCORES)])


def kernel_traced(Q, Wq, Wk, Wvin, Wvout, mask, clip):
    """Like kernel() but with NTFF tracing; returns (output, exec_time_ns)."""
    Mp, clipf, in_maps = _prep(Q, Wq, Wk, Wvin, Wvout, mask, clip)
    nc = _build(Mp, clipf)
    res = bass_utils.run_bass_kernel_spmd(
        nc, in_maps, core_ids=list(range(NCORES)), trace=True
    )
    out = np.stack([res.results[c]["O"] for c in range(NCORES)])
    return out, res.exec_time_ns


# revision 32
# speedup vs baseline: 1.1404x; 1.0283x over previous
"""Multi-head self-attention with tanh logit softcap + key masking, on 8 trn2 cores.

Strategy:
- Data-parallel over batch: core b handles batch b (B=8).
- Sparsity: masked keys (mask=True) are removed on the host; K/V are projected
  from a compacted Qc (unmasked rows only, zero-padded to M_pad). Pad columns
  are excluded from the softmax denominator via the padmask columns of the
  combined PV lhsT, and contribute zero to the numerator because their V rows
  are exactly zero.
- Head padding: the 8 heads (D=16) are padded to 32-row strips (2 groups of 4
  heads) via host-permuted projection weights, so score matmuls (K=16->32) can
  use 4-way tile_position row tiling and PV matmuls use col tiling.
- Scores are computed transposed (U^T[m, n]) so that P^T = exp(clip*tanh(U^T/4))
  feeds the PV matmuls (contraction over m on the partition dim) directly.
- PV lhsT = [v | padmask] (M=32): rows 0:16 of each strip accumulate the
  numerator, rows 16:32 the (replicated) denominator. An S matmul gathers the
  denominator rows and broadcasts their reciprocals to all 128 partitions for
  a single full-tile normalize multiply. The output projection uses
  host-padded Wvout rows (zeros kill the denominator/garbage rows).
- float32r (single-pass reduced-precision fp32) for the score/projection
  matmuls; PV matmuls stay fp32 (f32r rejects col-tiled tile_position).
"""

import functools
from contextlib import ExitStack

import numpy as np

import concourse.bass as bass
import concourse.mybir as mybir
import concourse.tile as tile
from concourse import bacc
from concourse import bass_utils
from concourse.tile import add_dep_helper

F32 = mybir.dt.float32
F32R = mybir.dt.float32r
AF = mybir.ActivationFunctionType

N = 1024
E = 128
H = 8
D = 16
NCORES = 8
NW = 512  # query-chunk width
NJ = N // NW  # number of query chunks

# S_BC[src, dst] = 1 iff src is the denominator row of dst's 32-strip:
# out[dst, n] = rdsb[32*(dst//32)+16, n] (gather+broadcast via matmul).
S_BC = np.zeros((128, 128), np.float32)
for _dst in range(128):
    S_BC[32 * (_dst // 32) + 16, _dst] = 1.0


def _pad_weight_in(W):
    """[E, H*D] -> [E, 2*128]: head h -> group h//4, strip 32*(h%4), cols 16:32 zero."""
    Wp = np.zeros((E, 2, 128), np.float32)
    for h in range(H):
        g, hp = divmod(h, 4)
        Wp[:, g, 32 * hp : 32 * hp + D] = W[:, D * h : D * (h + 1)]
    return np.ascontiguousarray(Wp.reshape(E, 256))


def _pad_weight_out(W):
    """[H*D, E] -> [128, 2*E] with padded rows (pad rows zero)."""
    Wp = np.zeros((128, 2, E), np.float32)
    for h in range(H):
        g, hp = divmod(h, 4)
        Wp[32 * hp : 32 * hp + D, g, :] = W[D * h : D * (h + 1), :]
    return np.ascontiguousarray(Wp.reshape(128, 2 * E))


@functools.lru_cache(maxsize=4)
def _build(Mp: int, clipf: float):
    MC = Mp // 128
    nc = bacc.Bacc(
        "TRN2",
        target_bir_lowering=False,
        debug=False,
        enable_asserts=True,
        num_devices=NCORES,
    )
    QT_d = nc.dram_tensor("QT", [E, N], F32R, kind="ExternalInput").ap()
    QcT_d = nc.dram_tensor("QcT", [E, Mp], F32R, kind="ExternalInput").ap()
    pm_d = nc.dram_tensor("pm32", [Mp, 32], F32, kind="ExternalInput").ap()
    wq_d = nc.dram_tensor("Wqp", [E, 256], F32R, kind="ExternalInput").ap()
    wk_d = nc.dram_tensor("Wkp", [E, 256], F32R, kind="ExternalInput").ap()
    wvi_d = nc.dram_tensor("Wvip", [E, 256], F32R, kind="ExternalInput").ap()
    wvo_d = nc.dram_tensor("Wvop", [128, 256], F32R, kind="ExternalInput").ap()
    O_d = nc.dram_tensor("O", [N, E], F32, kind="ExternalOutput").ap()

    with tile.TileContext(nc) as tc, ExitStack() as ctx:
        consts = ctx.enter_context(tc.tile_pool(name="consts", bufs=1))
        sbt = ctx.enter_context(tc.tile_pool(name="sbt", bufs=2))
        sbp = ctx.enter_context(tc.tile_pool(name="sbp", bufs=3))
        sbn = ctx.enter_context(tc.tile_pool(name="sbn", bufs=2))
        sbo = ctx.enter_context(tc.tile_pool(name="sbo", bufs=3))
        psq = ctx.enter_context(tc.tile_pool(name="psq", bufs=2, space="PSUM"))
        psp = ctx.enter_context(tc.tile_pool(name="psp", bufs=1, space="PSUM"))
        pvo = ctx.enter_context(tc.tile_pool(name="pvo", bufs=1, space="PSUM"))

        # ---------- loads ----------
        wtile = consts.tile([128, 1], F32)
        nc.vector.memset(wtile, 0.0)
        # warmup activation: forces the ACT table load to overlap input DMAs
        nc.scalar.activation(wtile, wtile, AF.Tanh, scale=0.25)

        wq = consts.tile([E, 256], F32R)
        nc.sync.dma_start(out=wq, in_=wq_d)
        wk = consts.tile([E, 256], F32R)
        nc.scalar.dma_start(out=wk, in_=wk_d)
        qt = consts.tile([E, N], F32R)
        nc.sync.dma_start(out=qt[:, 0:512], in_=QT_d[:, 0:512])
        qct = consts.tile([E, Mp], F32R)
        nc.scalar.dma_start(out=qct[:, 0 : min(512, Mp)], in_=QcT_d[:, 0 : min(512, Mp)])
        nc.sync.dma_start(out=qt[:, 512:], in_=QT_d[:, 512:])
        if Mp > 512:
            nc.scalar.dma_start(out=qct[:, 512:], in_=QcT_d[:, 512:])
        wvi = consts.tile([E, 256], F32R)
        nc.gpsimd.dma_start(out=wvi, in_=wvi_d)
        pm32 = consts.tile([128, MC, 32], F32)
        nc.gpsimd.dma_start(out=pm32, in_=pm_d.rearrange("(c p) w -> p c w", p=128))
        pm = consts.tile([128, MC, 128], F32)
        pmr = pm.rearrange("p c (s w) -> p c s w", s=4)
        for s4 in range(4):
            nc.vector.tensor_copy(pmr[:, :, s4, :], pm32)
        # S_BC[src, dst] = 1 iff src == 32*(dst//32)+16, built via affine_select
        sbc = consts.tile([128, 4, 32], F32)
        nc.gpsimd.memset(sbc, 0.0)
        nc.gpsimd.affine_select(
            out=sbc,
            in_=sbc,
            compare_op=mybir.AluOpType.not_equal,
            fill=1.0,
            base=-16,
            pattern=[[-32, 4], [0, 32]],
            channel_multiplier=1,
        )
        sbc = sbc.rearrange("p a b -> p (a b)")
        wvo = consts.tile([128, 256], F32R)
        nc.gpsimd.dma_start(out=wvo, in_=wvo_d)

        dz = consts.tile([128, NW], F32)
        nc.vector.memset(dz, 0.0)
        dzr = consts.tile([128, NW], F32R)
        nc.vector.tensor_copy(dzr, dz)

        qTp = [consts.tile([128, N], F32R, name=f"qTp{g}") for g in range(2)]
        kTp = [consts.tile([128, Mp], F32R, name=f"kTp{g}") for g in range(2)]
        vp = [consts.tile([128, MC, 128], F32, name=f"vp{g}") for g in range(2)]

        # ---------- projection emitters (interleaved into the main loop) ----------
        def emit_qproj(j):
            js = slice(j * 512, (j + 1) * 512)
            for g in range(2):
                gs = slice(g * 128, (g + 1) * 128)
                pq = psq.tile([128, 512], F32, tag="proj", name="pq")
                nc.tensor.matmul(pq, lhsT=wq[:, gs], rhs=qt[:, js], start=True, stop=True)
                nc.vector.tensor_copy(qTp[g][:, js], pq)

        def emit_kproj(s, w):
            for g in range(2):
                gs = slice(g * 128, (g + 1) * 128)
                pk = psq.tile([128, 512], F32, tag="proj", name="pk")
                nc.tensor.matmul(
                    pk[:, :w], lhsT=wk[:, gs], rhs=qct[:, s : s + w], start=True, stop=True
                )
                nc.vector.tensor_copy(kTp[g][:, s : s + w], pk[:, :w])

        def emit_vproj(c):
            cs = slice(c * 128, (c + 1) * 128)
            for g in range(2):
                gs = slice(g * 128, (g + 1) * 128)
                pv_ = psq.tile([128, 512], F32, tag="proj", name="pv_")
                nc.tensor.matmul(
                    pv_[:, :128], lhsT=qct[:, cs], rhs=wvi[:, gs], start=True, stop=True
                )
                nc.vector.tensor_add(vp[g][:, c, :], pv_[:, :128], pm[:, c, :])

        emit_qproj(0)
        emit_kproj(0, 128)
        emit_kproj(128, min(512, Mp) - 128)
        emit_vproj(0)
        # deferred prologue pieces, emitted after given (j, mc) iterations
        deferred = {}
        for c in range(1, MC):
            deferred.setdefault((0, c - 1), []).append(lambda c=c: emit_vproj(c))
        deferred.setdefault((0, 0), []).append(lambda: emit_qproj(1))
        if Mp > 512:
            deferred.setdefault((0, 1), []).append(lambda: emit_kproj(512, Mp - 512))

        # ---------- main loop over query chunks ----------
        o_accs = {}

        def emit_epilogue(j):
            o_acc = o_accs[j]
            osb = sbn.tile([128, 2, NW], F32, tag="osb", name="osb")
            nc.scalar.copy(osb, o_acc)
            rdsb = sbn.tile([128, 2, NW], F32, tag="rdsb", name="rdsb")
            nc.vector.reciprocal_approx_fast(out=rdsb, in_=o_acc)
            rd_b = pvo.tile([128, 2, NW], F32, tag="oacc", name="rd_b")
            for g in range(2):
                nc.tensor.matmul(
                    rd_b[:, g, :], lhsT=sbc, rhs=rdsb[:, g, :], start=True, stop=True
                )
            outn = sbn.tile([128, 2, NW], F32R, tag="outn", name="outn")
            nc.vector.tensor_mul(outn, osb, rd_b)

            for q in range(NW // 128):
                pf_full = pvo.tile([128, 2, NW], F32, tag="oacc", name="pf_full")
                pf = pf_full[:, 0, :128]
                pf0 = None
                for g in range(2):
                    mm = nc.tensor.matmul(
                        pf,
                        lhsT=outn[:, g, q * 128 : (q + 1) * 128],
                        rhs=wvo[:, g * 128 : (g + 1) * 128],
                        start=(g == 0),
                        stop=(g == 1),
                    )
                    if g == 0:
                        pf0 = mm
                    else:
                        add_dep_helper(
                            mm.ins, pf0.ins, sync=False, reason="proj accum order"
                        )
                fout = sbo.tile([128, 128], F32, tag="fout", name="fout")
                nc.vector.tensor_copy(fout, pf)
                nc.sync.dma_start(
                    out=O_d[j * NW + q * 128 : j * NW + (q + 1) * 128, :], in_=fout
                )

        for j in range(NJ):
            js = slice(j * NW, (j + 1) * NW)
            first_pv = {}

            for c in range(MC):
                cs = slice(c * 128, (c + 1) * 128)
                t = sbt.tile([128, 2, 4, NW], F32, tag="t", name="t")
                p = sbp.tile([128, 2, 4, NW], F32, tag="p", name="p")
                last = c == MC - 1
                for g in range(2):
                    ps = psp.tile([128, 4, NW], F32, tag="scores", name="ps")
                    for hp in range(4):
                        rs = slice(32 * hp, 32 * (hp + 1))
                        nc.tensor.matmul(
                            ps[:, hp, :],
                            lhsT=kTp[g][rs, cs],
                            rhs=qTp[g][rs, js],
                            start=True,
                            stop=True,
                            tile_position=(32 * hp, 0),
                        )
                    nc.scalar.activation(t[:, g], ps, AF.Tanh, scale=0.25)
                split_exp = c == MC - 1 and j == NJ - 1
                if not split_exp:
                    nc.scalar.activation(p, t, AF.Exp, scale=clipf)
                if c == 0:
                    # previous chunk's epilogue goes after this chunk's ACT work
                    # so ACT keeps streaming while PE runs the epilogue.
                    if j > 0:
                        emit_epilogue(j - 1)
                    o_acc = pvo.tile([128, 2, NW], F32, tag="oacc", name="o_acc")
                    o_accs[j] = o_acc
                    # dummy zeroing matmuls: start each psum bank's accumulation
                    # group with a full-partition write so the col-tiled M=32
                    # matmuls below can accumulate (has_written set everywhere).
                    for g in range(2):
                        first_pv[g] = nc.tensor.matmul(
                            o_acc[:, g, :], lhsT=dzr[:, 0:128], rhs=dzr, start=True,
                            stop=False, skip_group_check=True,
                        )
                for g in range(2):
                    if split_exp:
                        nc.scalar.activation(p[:, g], t[:, g], AF.Exp, scale=clipf)
                    for hp in range(4):
                        cols = slice(32 * hp, 32 * (hp + 1))
                        mo = nc.tensor.matmul(
                            o_acc[cols, g, :],
                            lhsT=vp[g][:, c, cols],
                            rhs=p[:, g, hp, :],
                            start=False,
                            stop=last,
                            tile_position=(0, 32 * hp),
                            skip_group_check=True,
                        )
                        add_dep_helper(
                            mo.ins,
                            first_pv[g].ins,
                            sync=False,
                            reason="psum group start",
                        )
                for fn in deferred.pop((j, c), []):
                    fn()

        emit_epilogue(NJ - 1)

    nc.compile()
    return nc


def _prep(Q, Wq, Wk, Wvin, Wvout, mask, clip):
    Q = np.asarray(Q, dtype=np.float32)
    Wq = np.asarray(Wq, dtype=np.float32)
    Wk = np.asarray(Wk, dtype=np.float32)
    Wvin = np.asarray(Wvin, dtype=np.float32)
    Wvout = np.asarray(Wvout, dtype=np.float32)
    mask = np.asarray(mask)
    clipf = float(np.asarray(clip))
    B = Q.shape[0]

    counts = (~mask).sum(axis=1)
    Mp = max(128, int(-(-int(counts.max()) // 128) * 128))

    wq_p = _pad_weight_in(Wq)
    wk_p = _pad_weight_in(Wk)
    wvi_p = _pad_weight_in(Wvin)
    wvo_p = _pad_weight_out(Wvout)

    in_maps = []
    for b in range(B):
        idx = np.nonzero(~mask[b])[0]
        Qc = np.zeros((Mp, E), np.float32)
        Qc[: len(idx)] = Q[b, idx]
        pmv = np.zeros((Mp, 1), np.float32)
        pmv[: len(idx)] = 1.0
        pm32 = np.zeros((Mp, 32), np.float32)
        pm32[:, 16:32] = pmv
        in_maps.append(
            {
                "QT": np.ascontiguousarray(Q[b].T),
                "QcT": np.ascontiguousarray(Qc.T),
                "pm32": pm32,
                "Wqp": wq_p,
                "Wkp": wk_p,
                "Wvip": wvi_p,
                "Wvop": wvo_p,
            }
        )
    return Mp, clipf, in_maps


def kernel(Q, Wq, Wk, Wvin, Wvout, mask, clip):
    Mp, clipf, in_maps = _prep(Q, Wq, Wk, Wvin, Wvout, mask, clip)
    nc = _build(Mp, clipf)
    res = bass_utils.run_bass_kernel_spmd(nc, in_maps, core_ids=list(range(NCORES)))
    return np.stack([res.results[c]["O"] for c in range(NCORES)])


def kernel_traced(Q, Wq, Wk, Wvin, Wvout, mask, clip):
    """Like kernel() but with NTFF tracing; returns (output, exec_time_ns)."""
    Mp, clipf, in_maps = _prep(Q, Wq, Wk, Wvin, Wvout, mask, clip)
    nc = _build(Mp, clipf)
    res = bass_utils.run_bass_kernel_spmd(
        nc, in_maps, core_ids=list(range(NCORES)), trace=True
    )
    out = np.stack([res.results[c]["O"] for c in range(NCORES)])
    return out, res.exec_time_ns


# revision 33
# speedup vs baseline: 1.2775x; 1.1202x over previous
"""Multi-head self-attention with tanh logit softcap + key masking, on 8 trn2 cores.

Strategy:
- Data-parallel over batch: core b handles batch b (B=8).
- Sparsity: masked keys (mask=True) are removed on the host; K/V are projected
  from a compacted Qc (unmasked rows only, zero-padded to M_pad). Pad columns
  are excluded from the softmax denominator via the padmask columns of the
  combined PV lhsT, and contribute zero to the numerator because their V rows
  are exactly zero.
- Head padding: the 8 heads (D=16) are padded to 32-row strips (2 groups of 4
  heads) via host-permuted projection weights, so score matmuls (K=16->32) can
  use 4-way tile_position row tiling and PV matmuls use col tiling.
- Scores are computed transposed (U^T[m, n]) so that P^T = exp(clip*tanh(U^T/4))
  feeds the PV matmuls (contraction over m on the partition dim) directly.
- PV lhsT = [v | padmask] (M=32): rows 0:16 of each strip accumulate the
  numerator, rows 16:32 the (replicated) denominator. An S matmul gathers the
  denominator rows and broadcasts their reciprocals to all 128 partitions for
  a single full-tile normalize multiply. The output projection uses
  host-padded Wvout rows (zeros kill the denominator/garbage rows).
- float32r (single-pass reduced-precision fp32) for the score/projection
  matmuls; PV matmuls stay fp32 (f32r rejects col-tiled tile_position).
"""

import functools
from contextlib import ExitStack

import numpy as np

import concourse.bass as bass
import concourse.mybir as mybir
import concourse.tile as tile
from concourse import bacc
from concourse import bass_utils
from concourse.tile import add_dep_helper

F32 = mybir.dt.float32
F32R = mybir.dt.float32r
BF16 = mybir.dt.bfloat16
AF = mybir.ActivationFunctionType

N = 1024
E = 128
H = 8
D = 16
NCORES = 8
NW = 512  # query-chunk width
NJ = N // NW  # number of query chunks

# S_BC[src, dst] = 1 iff src is the denominator row of dst's 32-strip:
# out[dst, n] = rdsb[32*(dst//32)+16, n] (gather+broadcast via matmul).
S_BC = np.zeros((128, 128), np.float32)
for _dst in range(128):
    S_BC[32 * (_dst // 32) + 16, _dst] = 1.0


def _pad_weight_in(W):
    """[E, H*D] -> [E, 2*128]: head h -> group h//4, strip 32*(h%4), cols 16:32 zero."""
    Wp = np.zeros((E, 2, 128), np.float32)
    for h in range(H):
        g, hp = divmod(h, 4)
        Wp[:, g, 32 * hp : 32 * hp + D] = W[:, D * h : D * (h + 1)]
    return np.ascontiguousarray(Wp.reshape(E, 256))


def _pad_weight_out(W):
    """[H*D, E] -> [128, 2*E] with padded rows (pad rows zero)."""
    Wp = np.zeros((128, 2, E), np.float32)
    for h in range(H):
        g, hp = divmod(h, 4)
        Wp[32 * hp : 32 * hp + D, g, :] = W[D * h : D * (h + 1), :]
    return np.ascontiguousarray(Wp.reshape(128, 2 * E))


@functools.lru_cache(maxsize=4)
def _build(Mp: int, clipf: float):
    MC = Mp // 128
    nc = bacc.Bacc(
        "TRN2",
        target_bir_lowering=False,
        debug=False,
        enable_asserts=True,
        num_devices=NCORES,
    )
    QT_d = nc.dram_tensor("QT", [E, N], F32R, kind="ExternalInput").ap()
    QcT_d = nc.dram_tensor("QcT", [E, Mp], F32R, kind="ExternalInput").ap()
    pm_d = nc.dram_tensor("pm32", [Mp, 32], F32, kind="ExternalInput").ap()
    wq_d = nc.dram_tensor("Wqp", [E, 256], F32R, kind="ExternalInput").ap()
    wk_d = nc.dram_tensor("Wkp", [E, 256], F32R, kind="ExternalInput").ap()
    wvi_d = nc.dram_tensor("Wvip", [E, 256], F32R, kind="ExternalInput").ap()
    wvo_d = nc.dram_tensor("Wvop", [128, 256], F32R, kind="ExternalInput").ap()
    O_d = nc.dram_tensor("O", [N, E], F32, kind="ExternalOutput").ap()

    with tile.TileContext(nc) as tc, ExitStack() as ctx:
        consts = ctx.enter_context(tc.tile_pool(name="consts", bufs=1))
        sbt = ctx.enter_context(tc.tile_pool(name="sbt", bufs=2))
        sbp = ctx.enter_context(tc.tile_pool(name="sbp", bufs=3))
        sbn = ctx.enter_context(tc.tile_pool(name="sbn", bufs=2))
        sbo = ctx.enter_context(tc.tile_pool(name="sbo", bufs=3))
        psq = ctx.enter_context(tc.tile_pool(name="psq", bufs=2, space="PSUM"))
        psp = ctx.enter_context(tc.tile_pool(name="psp", bufs=1, space="PSUM"))
        pvo = ctx.enter_context(tc.tile_pool(name="pvo", bufs=1, space="PSUM"))

        # ---------- loads ----------
        wtile = consts.tile([128, 1], F32)
        nc.vector.memset(wtile, 0.0)
        # warmup activation: forces the ACT table load to overlap input DMAs
        nc.scalar.activation(wtile, wtile, AF.Tanh, scale=0.25)

        wq = consts.tile([E, 256], F32R)
        nc.sync.dma_start(out=wq, in_=wq_d)
        wk = consts.tile([E, 256], F32R)
        nc.scalar.dma_start(out=wk, in_=wk_d)
        qt = consts.tile([E, N], F32R)
        nc.sync.dma_start(out=qt[:, 0:512], in_=QT_d[:, 0:512])
        qct = consts.tile([E, Mp], F32R)
        nc.scalar.dma_start(out=qct[:, 0 : min(512, Mp)], in_=QcT_d[:, 0 : min(512, Mp)])
        nc.sync.dma_start(out=qt[:, 512:], in_=QT_d[:, 512:])
        if Mp > 512:
            nc.scalar.dma_start(out=qct[:, 512:], in_=QcT_d[:, 512:])
        wvi = consts.tile([E, 256], F32R)
        nc.gpsimd.dma_start(out=wvi, in_=wvi_d)
        pm32 = consts.tile([128, MC, 32], F32)
        nc.gpsimd.dma_start(out=pm32, in_=pm_d.rearrange("(c p) w -> p c w", p=128))
        pm = consts.tile([128, MC, 128], F32)
        pmr = pm.rearrange("p c (s w) -> p c s w", s=4)
        for s4 in range(4):
            nc.vector.tensor_copy(pmr[:, :, s4, :], pm32)
        # S_BC[src, dst] = 1 iff src == 32*(dst//32)+16, built via affine_select
        sbc = consts.tile([128, 4, 32], F32)
        nc.gpsimd.memset(sbc, 0.0)
        nc.gpsimd.affine_select(
            out=sbc,
            in_=sbc,
            compare_op=mybir.AluOpType.not_equal,
            fill=1.0,
            base=-16,
            pattern=[[-32, 4], [0, 32]],
            channel_multiplier=1,
        )
        sbc = sbc.rearrange("p a b -> p (a b)")
        wvo = consts.tile([128, 256], F32R)
        nc.gpsimd.dma_start(out=wvo, in_=wvo_d)

        dz = consts.tile([128, NW], F32)
        nc.vector.memset(dz, 0.0)
        dzr = consts.tile([128, NW], F32R)
        nc.vector.tensor_copy(dzr, dz)
        dzb = consts.tile([128, NW], BF16)
        nc.vector.tensor_copy(dzb, dz)

        qTp = [consts.tile([128, N], F32R, name=f"qTp{g}") for g in range(2)]
        kTp = [consts.tile([128, Mp], F32R, name=f"kTp{g}") for g in range(2)]
        vp = [consts.tile([128, MC, 128], BF16, name=f"vp{g}") for g in range(2)]

        # ---------- projection emitters (interleaved into the main loop) ----------
        def emit_qproj(j):
            js = slice(j * 512, (j + 1) * 512)
            for g in range(2):
                gs = slice(g * 128, (g + 1) * 128)
                pq = psq.tile([128, 512], F32, tag="proj", name="pq")
                nc.tensor.matmul(pq, lhsT=wq[:, gs], rhs=qt[:, js], start=True, stop=True)
                nc.vector.tensor_copy(qTp[g][:, js], pq)

        def emit_kproj(s, w):
            for g in range(2):
                gs = slice(g * 128, (g + 1) * 128)
                pk = psq.tile([128, 512], F32, tag="proj", name="pk")
                nc.tensor.matmul(
                    pk[:, :w], lhsT=wk[:, gs], rhs=qct[:, s : s + w], start=True, stop=True
                )
                nc.vector.tensor_copy(kTp[g][:, s : s + w], pk[:, :w])

        def emit_vproj(c):
            cs = slice(c * 128, (c + 1) * 128)
            for g in range(2):
                gs = slice(g * 128, (g + 1) * 128)
                pv_ = psq.tile([128, 512], F32, tag="proj", name="pv_")
                nc.tensor.matmul(
                    pv_[:, :128], lhsT=qct[:, cs], rhs=wvi[:, gs], start=True, stop=True
                )
                nc.vector.tensor_add(vp[g][:, c, :], pv_[:, :128], pm[:, c, :])

        emit_qproj(0)
        emit_kproj(0, 128)
        emit_kproj(128, min(512, Mp) - 128)
        emit_vproj(0)
        # deferred prologue pieces, emitted after given (j, mc) iterations
        deferred = {}
        for c in range(1, MC):
            deferred.setdefault((0, c - 1), []).append(lambda c=c: emit_vproj(c))
        deferred.setdefault((0, 0), []).append(lambda: emit_qproj(1))
        if Mp > 512:
            deferred.setdefault((0, 1), []).append(lambda: emit_kproj(512, Mp - 512))

        # ---------- main loop over query chunks ----------
        o_accs = {}

        def emit_epilogue(j):
            o_acc = o_accs[j]
            osb = sbn.tile([128, 2, NW], F32, tag="osb", name="osb")
            nc.scalar.copy(osb, o_acc)
            rdsb = sbn.tile([128, 2, NW], F32, tag="rdsb", name="rdsb")
            nc.vector.reciprocal_approx_fast(out=rdsb, in_=o_acc)
            rd_b = pvo.tile([128, 2, NW], F32, tag="oacc", name="rd_b")
            for g in range(2):
                nc.tensor.matmul(
                    rd_b[:, g, :], lhsT=sbc, rhs=rdsb[:, g, :], start=True, stop=True
                )
            outn = sbn.tile([128, 2, NW], F32R, tag="outn", name="outn")
            nc.vector.tensor_mul(outn, osb, rd_b)

            for q in range(NW // 128):
                pf_full = pvo.tile([128, 2, NW], F32, tag="oacc", name="pf_full")
                pf = pf_full[:, 0, :128]
                pf0 = None
                for g in range(2):
                    mm = nc.tensor.matmul(
                        pf,
                        lhsT=outn[:, g, q * 128 : (q + 1) * 128],
                        rhs=wvo[:, g * 128 : (g + 1) * 128],
                        start=(g == 0),
                        stop=(g == 1),
                    )
                    if g == 0:
                        pf0 = mm
                    else:
                        add_dep_helper(
                            mm.ins, pf0.ins, sync=False, reason="proj accum order"
                        )
                fout = sbo.tile([128, 128], F32, tag="fout", name="fout")
                nc.vector.tensor_copy(fout, pf)
                nc.sync.dma_start(
                    out=O_d[j * NW + q * 128 : j * NW + (q + 1) * 128, :], in_=fout
                )

        for j in range(NJ):
            js = slice(j * NW, (j + 1) * NW)
            first_pv = {}

            for c in range(MC):
                cs = slice(c * 128, (c + 1) * 128)
                t = sbt.tile([128, 2, 4, NW], F32, tag="t", name="t")
                p = sbp.tile([128, 2, 4, NW], BF16, tag="p", name="p")
                last = c == MC - 1
                for g in range(2):
                    ps = psp.tile([128, 4, NW], F32, tag="scores", name="ps")
                    for hp in range(4):
                        rs = slice(32 * hp, 32 * (hp + 1))
                        nc.tensor.matmul(
                            ps[:, hp, :],
                            lhsT=kTp[g][rs, cs],
                            rhs=qTp[g][rs, js],
                            start=True,
                            stop=True,
                            tile_position=(32 * hp, 0),
                        )
                    nc.scalar.activation(t[:, g], ps, AF.Tanh, scale=0.25)
                split_exp = c == MC - 1 and j == NJ - 1
                if not split_exp:
                    nc.scalar.activation(p, t, AF.Exp, scale=clipf)
                if c == 0:
                    # previous chunk's epilogue goes after this chunk's ACT work
                    # so ACT keeps streaming while PE runs the epilogue.
                    if j > 0:
                        emit_epilogue(j - 1)
                    o_acc = pvo.tile([128, 2, NW], F32, tag="oacc", name="o_acc")
                    o_accs[j] = o_acc
                    # dummy zeroing matmuls: start each psum bank's accumulation
                    # group with a full-partition write so the col-tiled M=32
                    # matmuls below can accumulate (has_written set everywhere).
                    for g in range(2):
                        first_pv[g] = nc.tensor.matmul(
                            o_acc[:, g, :], lhsT=dzb[:, 0:128], rhs=dzb, start=True,
                            stop=False, skip_group_check=True,
                        )
                for g in range(2):
                    if split_exp:
                        nc.scalar.activation(p[:, g], t[:, g], AF.Exp, scale=clipf)
                    for hp in range(4):
                        cols = slice(32 * hp, 32 * (hp + 1))
                        mo = nc.tensor.matmul(
                            o_acc[cols, g, :],
                            lhsT=vp[g][:, c, cols],
                            rhs=p[:, g, hp, :],
                            start=False,
                            stop=last,
                            tile_position=(0, 32 * hp),
                            skip_group_check=True,
                        )
                        add_dep_helper(
                            mo.ins,
                            first_pv[g].ins,
                            sync=False,
                            reason="psum group start",
                        )
                for fn in deferred.pop((j, c), []):
                    fn()

        emit_epilogue(NJ - 1)

    nc.compile()
    return nc


def _prep(Q, Wq, Wk, Wvin, Wvout, mask, clip):
    Q = np.asarray(Q, dtype=np.float32)
    Wq = np.asarray(Wq, dtype=np.float32)
    Wk = np.asarray(Wk, dtype=np.float32)
    Wvin = np.asarray(Wvin, dtype=np.float32)
    Wvout = np.asarray(Wvout, dtype=np.float32)
    mask = np.asarray(mask)
    clipf = float(np.asarray(clip))
    B = Q.shape[0]

    counts = (~mask).sum(axis=1)
    Mp = max(128, int(-(-int(counts.max()) // 128) * 128))

    wq_p = _pad_weight_in(Wq)
    wk_p = _pad_weight_in(Wk)
    wvi_p = _pad_weight_in(Wvin)
    wvo_p = _pad_weight_out(Wvout)

    in_maps = []
    for b in range(B):
        idx = np.nonzero(~mask[b])[0]
        Qc = np.zeros((Mp, E), np.float32)
        Qc[: len(idx)] = Q[b, idx]
        pmv = np.zeros((Mp, 1), np.float32)
        pmv[: len(idx)] = 1.0
        pm32 = np.zeros((Mp, 32), np.float32)
        pm32[:, 16:32] = pmv
        in_maps.append(
            {
                "QT": np.ascontiguousarray(Q[b].T),
                "QcT": np.ascontiguousarray(Qc.T),
                "pm32": pm32,
                "Wqp": wq_p,
                "Wkp": wk_p,
                "Wvip": wvi_p,
                "Wvop": wvo_p,
            }
        )
    return Mp, clipf, in_maps


def kernel(Q, Wq, Wk, Wvin, Wvout, mask, clip):
    Mp, clipf, in_maps = _prep(Q, Wq, Wk, Wvin, Wvout, mask, clip)
    nc = _build(Mp, clipf)
    res = bass_utils.run_bass_kernel_spmd(nc, in_maps, core_ids=list(range(NCORES)))
    return np.stack([res.results[c]["O"] for c in range(NCORES)])


def kernel_traced(Q, Wq, Wk, Wvin, Wvout, mask, clip):
    """Like kernel() but with NTFF tracing; returns (output, exec_time_ns)."""
    Mp, clipf, in_maps = _prep(Q, Wq, Wk, Wvin, Wvout, mask, clip)
    nc = _build(Mp, clipf)
    res = bass_utils.run_bass_kernel_spmd(
        nc, in_maps, core_ids=list(range(NCORES)), trace=True
    )
    out = np.stack([res.results[c]["O"] for c in range(NCORES)])
    return out, res.exec_time_ns
